# revision 1
# baseline (speedup 1.0000x reference)
"""GCN autoencoder (4x GCNConv) on 8 Trainium2 NeuronCores.

Strategy
--------
Math identity: with A' = A + I (self loops) and D = diag(deg),
    gcn(h) = D^-1/2 A' D^-1/2 (h @ W) + b
and aggregation commutes with the dense transform, so every layer is run
"transform first":  z = dinv * (h @ W)  (node-wise scale), then an
unweighted gather/segment-sum over the fixed edge list, then a second
dinv scale, bias and (optionally) relu.  The per-edge norm tensor is
never materialized.

Device mapping (per core, nodes sharded 12500/core, padded to 12544 =
128*98):
  * z is computed node-major via PE (per-128-node-tile transpose +
    matmul), scaled by dinv, cast to fp16 and stored as 256B rows
    [N, 128] in DRAM; an AllGather replicates the full table.
  * Edges (sharded by dst core, plus self loops) are fetched with
    dma_gather.  int16 gather indices only address 32K rows, so the
    gid space (core*12544 + local) is split into 4 quartiles of 25088
    rows; quartile q's gathers run on SWDGE queue q (desc generation
    parallelism across Q7 core pairs - measured 2.2ns/row aggregate).
  * Segment-sum runs on the PE: each 128-edge chunk is multiplied by a
    host-built one-hot fp16 selector [128 slots, 128 dst] accumulating
    into the dst tile's PSUM bank (fp32).  Selector streams are pure
    input data, so per-core ragged edge counts hide behind a shared
    compiled schedule (chunk counts are the max over cores).
  * PSUM evac applies dinv (fp32), then bias add + relu.

Everything dense stays fp32; only the gathered payload (z) is fp16
(~2e-4 relative error on the aggregate, fp32 accumulation in PSUM).
"""

import math

import numpy as np

import concourse.bacc as bacc
import concourse.bass as bass
import concourse.mybir as mybir
import concourse.tile as tile
from concourse.bass_utils import run_bass_kernel_spmd
from concourse.masks import make_identity

P = 128
NC = 8
NQ = 4  # SWDGE queues == src-gid quartiles

# layer schedule: (F_in, F_out, relu)
LAYERS = [(88, 65, True), (65, 50, False), (50, 65, True), (65, 88, False)]
FMAX = 128  # fp16 z-table row width (256B rows)

last_results = None  # stashed BassKernelResults for test harnesses


def _build_host_data(x, Ws, bs, edge_index, n_real, t_tiles):
    """Shard + schedule. Returns per-core inputs and the shared schedule."""
    npc_real = n_real // NC          # real nodes per core
    npc = P * t_tiles                # padded nodes per core
    n_pad_total = NC * npc
    quart = n_pad_total // NQ        # 25088 for full size
    assert quart <= 32767, "quartile must stay int16-addressable"

    src = edge_index[0].astype(np.int64)
    dst = edge_index[1].astype(np.int64)

    deg = np.bincount(dst, minlength=n_real).astype(np.float32) + 1.0

    # core/local/tile/partition of every node id
    def core_of(n):
        return n // npc_real

    def local_of(n):
        return n - (n // npc_real) * npc_real

    # gid = core*npc + local ; tile = local % T ; p = local // T
    gid_src = core_of(src) * npc + local_of(src)
    c_dst = core_of(dst)
    r_dst = local_of(dst)

    # self loops
    all_n = np.arange(n_real, dtype=np.int64)
    gid_src = np.concatenate([gid_src, core_of(all_n) * npc + local_of(all_n)])
    c_dst = np.concatenate([c_dst, core_of(all_n)])
    r_dst = np.concatenate([r_dst, local_of(all_n)])

    t_dst = r_dst % t_tiles
    p_dst = r_dst // t_tiles
    q_src = gid_src // quart
    idx_local = (gid_src - q_src * quart).astype(np.int64)

    # group edges by (core, tile, quartile)
    key = ((c_dst * t_tiles + t_dst) * NQ + q_src).astype(np.int64)
    order = np.argsort(key, kind="stable")
    key_s = key[order]
    idx_s = idx_local[order]
    p_s = p_dst[order]

    nkeys = NC * t_tiles * NQ
    counts = np.bincount(key_s, minlength=nkeys).reshape(NC, t_tiles, NQ)
    starts = np.zeros(nkeys + 1, np.int64)
    np.cumsum(counts.reshape(-1), out=starts[1:])

    # shared schedule: chunks per (tile, quartile) = ceil(max_c count / 128)
    max_counts = counts.max(axis=0)  # [T, NQ]
    n_chunk = np.ceil(max_counts / P).astype(np.int64)  # [T, NQ]

    # gather call groups: tiles grouped in G=5 per quartile
    G = 5
    groups = [list(range(g, min(g + G, t_tiles))) for g in range(0, t_tiles, G)]

    # per (g,q): num idxs
    nidx_gq = [[int(n_chunk[g, :][:, q].sum() * P) for q in range(NQ)] for g in groups]
    for row in nidx_gq:
        for v in row:
            assert v <= 16000, "gather call too large for uint16/ring"

    # chunk ids laid out by (t, q, c) for the selector stream
    chunk_off = np.zeros((t_tiles, NQ), np.int64)
    acc = 0
    for t in range(t_tiles):
        for q in range(NQ):
            chunk_off[t, q] = acc
            acc += n_chunk[t, q]
    nch_tot = int(acc)

    zero_row_local = npc_real  # pad row inside every quartile's first core

    idx_cols_tot = sum(sum(row) for row in nidx_gq) // 16
    sel_width = nch_tot * P

    per_core = []
    for c in range(NC):
        idx_all = np.zeros((P, idx_cols_tot), np.int16)
        sel = np.zeros((P, sel_width), np.float16)
        col = 0
        for gi, g in enumerate(groups):
            for q in range(NQ):
                nidx = nidx_gq[gi][q]
                if nidx == 0:
                    continue
                vals = np.full(nidx, zero_row_local, np.int64)
                off = 0
                for t in g:
                    nct = int(n_chunk[t, q])
                    if nct == 0:
                        continue
                    k = key_from = (c * t_tiles + t) * NQ + q
                    s0, s1 = starts[key_from], starts[k + 1]
                    cnt = s1 - s0
                    vals[off : off + cnt] = idx_s[s0:s1]
                    # selector one-hot entries for this (t,q)
                    ch0 = chunk_off[t, q]
                    slots = np.arange(cnt)
                    sel_cols = (ch0 + slots // P) * P + p_s[s0:s1]
                    sel[slots % P, sel_cols] = np.float16(1.0)
                    off += nct * P
                assert off == nidx
                wrapped = vals.reshape(-1, 16).T.astype(np.int16)  # [16, nidx/16]
                idx_all[:, col : col + nidx // 16] = np.tile(wrapped, (8, 1))
                col += nidx // 16
        assert col == idx_cols_tot

        # node-major inputs in (p, t) layout
        xs = np.zeros((npc, x.shape[1]), np.float32)
        xs[:npc_real] = x[c * npc_real : (c + 1) * npc_real]
        degs = np.full(npc, 1.0e30, np.float32)
        degs[:npc_real] = deg[c * npc_real : (c + 1) * npc_real]
        # r = p*T + t  ->  [P, T] arrays
        x_pt = xs.reshape(P, t_tiles, x.shape[1]).reshape(P, t_tiles * x.shape[1])
        deg_pt = degs.reshape(P, t_tiles)

        inp = {"x": x_pt, "deg": deg_pt, "idx": idx_all, "sel": sel}
        for li in range(4):
            inp[f"W{li + 1}"] = Ws[li]
            inp[f"b{li + 1}"] = np.tile(bs[li].reshape(1, -1), (P, 1))
        per_core.append(inp)

    sched = {
        "t_tiles": t_tiles,
        "npc": npc,
        "npc_real": npc_real,
        "quart": quart,
        "groups": groups,
        "nidx_gq": nidx_gq,
        "n_chunk": n_chunk,
        "chunk_off": chunk_off,
        "nch_tot": nch_tot,
        "idx_cols_tot": idx_cols_tot,
        "fin0": x.shape[1],
    }
    return per_core, sched


def _build_program(sched):
    t_tiles = sched["t_tiles"]
    npc = sched["npc"]
    quart = sched["quart"]
    groups = sched["groups"]
    nidx_gq = sched["nidx_gq"]
    n_chunk = sched["n_chunk"]
    chunk_off = sched["chunk_off"]
    nch_tot = sched["nch_tot"]
    idx_cols_tot = sched["idx_cols_tot"]
    fin0 = sched["fin0"]
    fout_last = LAYERS[-1][1]
    n_pad_total = NC * npc

    f32, f16, i16 = mybir.dt.float32, mybir.dt.float16, mybir.dt.int16
    AF = mybir.ActivationFunctionType

    nc = bacc.Bacc(
        "TRN2",
        target_bir_lowering=False,
        debug=False,
        num_devices=NC,
        num_swdge_queues=NQ,
    )

    x_t = nc.dram_tensor("x", [P, t_tiles * fin0], f32, kind="ExternalInput")
    deg_t = nc.dram_tensor("deg", [P, t_tiles], f32, kind="ExternalInput")
    idx_t = nc.dram_tensor("idx", [P, idx_cols_tot], i16, kind="ExternalInput")
    sel_t = nc.dram_tensor("sel", [P, nch_tot * P], f16, kind="ExternalInput")
    W_ts, b_ts = [], []
    for li, (fi, fo, _) in enumerate(LAYERS):
        W_ts.append(nc.dram_tensor(f"W{li + 1}", [fi, fo], f32, kind="ExternalInput"))
        b_ts.append(nc.dram_tensor(f"b{li + 1}", [P, fo], f32, kind="ExternalInput"))
    out_t = nc.dram_tensor("out", [P, t_tiles * fout_last], f32, kind="ExternalOutput")

    zloc = nc.dram_tensor("zloc", [P, t_tiles * FMAX], f16)
    zfull = nc.dram_tensor("zfull", [n_pad_total, FMAX], f16, addr_space="Shared")

    with tile.TileContext(nc) as tc:
        with (
            tc.tile_pool(name="const", bufs=1) as cpool,
            tc.tile_pool(name="hbuf", bufs=1) as hpool,
            tc.tile_pool(name="zbuf", bufs=1) as zpool,
            tc.tile_pool(name="work", bufs=3) as wpool,
            tc.tile_pool(name="selp", bufs=3) as selpool,
            tc.tile_pool(name="gt", bufs=2) as gtpool,
            tc.tile_pool(name="ps", bufs=2, space="PSUM") as pspool,
            tc.tile_pool(name="agg", bufs=4, space="PSUM") as aggpool,
        ):
            ident = cpool.tile([P, P], f32)
            make_identity(nc, ident[:])

            idx_sb = cpool.tile([P, idx_cols_tot], i16)
            nc.sync.dma_start(out=idx_sb[:], in_=idx_t[:])

            deg_sb = cpool.tile([P, t_tiles], f32)
            nc.sync.dma_start(out=deg_sb[:], in_=deg_t[:])
            dinv = cpool.tile([P, t_tiles], f32)
            # dinv = 1/sqrt(deg): ACT sqrt then DVE reciprocal
            nc.scalar.activation(dinv[:], deg_sb[:], AF.Sqrt)
            nc.vector.reciprocal(dinv[:], dinv[:])

            Wsb, bsb = [], []
            for li, (fi, fo, _) in enumerate(LAYERS):
                w = cpool.tile([fi, fo], f32, tag=f"w{li}")
                nc.sync.dma_start(out=w[:], in_=W_ts[li][:])
                b = cpool.tile([P, fo], f32, tag=f"bi{li}")
                nc.sync.dma_start(out=b[:], in_=b_ts[li][:])
                Wsb.append(w)
                bsb.append(b)

            hbuf = hpool.tile([P, t_tiles, 88], f32)
            zbuf = zpool.tile([P, t_tiles, FMAX], f16)

            for li, (fi, fo, do_relu) in enumerate(LAYERS):
                # ---- z phase: z = dinv * (h @ W) in fp16, node major ----
                for t in range(t_tiles):
                    if li == 0:
                        xt = wpool.tile([P, fin0], f32, tag="xt")
                        nc.sync.dma_start(
                            out=xt[:], in_=x_t[:, t * fin0 : (t + 1) * fin0]
                        )
                        h_tile = xt[:, :fi]
                    else:
                        h_tile = hbuf[:, t, :fi]
                    ht_ps = pspool.tile([fi, P], f32, tag="tps")
                    nc.tensor.transpose(out=ht_ps[:], in_=h_tile, identity=ident[:])
                    ht = wpool.tile([fi, P], f32, tag="ht")
                    nc.scalar.activation(ht[:], ht_ps[:], AF.Copy)
                    z_ps = pspool.tile([P, fo], f32, tag="zps")
                    nc.tensor.matmul(
                        out=z_ps[:], lhsT=ht[:], rhs=Wsb[li][:], start=True, stop=True
                    )
                    nc.scalar.activation(
                        zbuf[:, t, :fo], z_ps[:], AF.Copy, scale=dinv[:, t : t + 1]
                    )

                nc.sync.dma_start(
                    out=zloc[:], in_=zbuf[:].rearrange("p t f -> p (t f)")
                )
                nc.gpsimd.collective_compute(
                    "AllGather",
                    mybir.AluOpType.bypass,
                    ins=[zloc[:]],
                    outs=[zfull[:]],
                    replica_groups=[list(range(NC))],
                )

                # ---- gather + selector-matmul segment sum ----
                col16 = 0
                for gi, g in enumerate(groups):
                    gts = []
                    for q in range(NQ):
                        nidx = nidx_gq[gi][q]
                        if nidx == 0:
                            gts.append(None)
                            continue
                        gt = gtpool.tile([P, nidx // P, FMAX], f16, tag=f"gt{q}")
                        nc.gpsimd.dma_gather(
                            out_ap=gt[:],
                            in_ap=zfull[q * quart : (q + 1) * quart, :],
                            idxs_ap=idx_sb[:, col16 : col16 + nidx // 16],
                            num_idxs=nidx,
                            num_idxs_reg=nidx,
                            elem_size=FMAX,
                            single_packet=False,
                            queue_num=q,
                        )
                        col16 += nidx // 16
                        gts.append(gt)
                    for t in g:
                        ncht = int(n_chunk[t, :].sum())
                        if ncht == 0:
                            continue
                        ch0 = int(chunk_off[t, 0])
                        selt = selpool.tile([P, ncht * P], f16, tag="sel")
                        nc.sync.dma_start(
                            out=selt[:],
                            in_=sel_t[:, ch0 * P : (ch0 + ncht) * P],
                        )
                        agg = aggpool.tile([P, fo], f32, tag="agg")
                        done = 0
                        for q in range(NQ):
                            nct = int(n_chunk[t, q])
                            if nct == 0:
                                continue
                            gt = gts[q]
                            # column offset of tile t inside this call
                            coff = int(
                                sum(int(n_chunk[tt, q]) for tt in g if tt < t)
                            )
                            for ci in range(nct):
                                sloc = int(chunk_off[t, q]) - ch0 + ci
                                nc.tensor.matmul(
                                    out=agg[:],
                                    lhsT=selt[:, sloc * P : (sloc + 1) * P],
                                    rhs=gt[:, coff + ci, :fo],
                                    start=(done == 0),
                                    stop=(done == ncht - 1),
                                )
                                done += 1
                        assert done == ncht
                        # ---- evac: h = relu(dinv*agg + b) ----
                        hslice = hbuf[:, t, :fo]
                        nc.scalar.activation(
                            hslice, agg[:], AF.Copy, scale=dinv[:, t : t + 1]
                        )
                        nc.vector.tensor_tensor(
                            out=hslice,
                            in0=hslice,
                            in1=bsb[li][:],
                            op=mybir.AluOpType.add,
                        )
                        if do_relu:
                            nc.scalar.activation(hslice, hslice, AF.Relu)
                assert col16 == idx_cols_tot

            nc.sync.dma_start(
                out=out_t[:],
                in_=hbuf[:, :, :fout_last].rearrange("p t f -> p (t f)"),
            )

    nc.compile()
    return nc


def kernel(x, W1, b1, W2, b2, W3, b3, W4, b4, edge_index):
    global last_results
    x = np.asarray(x, np.float32)
    edge_index = np.asarray(edge_index)
    n_real = x.shape[0]
    t_tiles = math.ceil(n_real / NC / P)  # 98 for 100000
    Ws = [np.asarray(w, np.float32) for w in (W1, W2, W3, W4)]
    bs = [np.asarray(b, np.float32) for b in (b1, b2, b3, b4)]

    per_core, sched = _build_host_data(x, Ws, bs, edge_index, n_real, t_tiles)
    nc = _build_program(sched)

    res = run_bass_kernel_spmd(nc, per_core, list(range(NC)))
    last_results = res

    npc_real = sched["npc_real"]
    fo = LAYERS[-1][1]
    out = np.empty((n_real, fo), np.float32)
    for c in range(NC):
        o = res.results[c]["out"].reshape(P, t_tiles, fo)
        out[c * npc_real : (c + 1) * npc_real] = o.reshape(P * t_tiles, fo)[:npc_real]
    return out



# revision 8
# speedup vs baseline: 1.7952x; 1.7952x over previous
"""GCN autoencoder (4x GCNConv) on 8 Trainium2 NeuronCores.

Strategy (v2)
-------------
Math identity: with A' = A + I and D = diag(deg),
    gcn(h) = D^-1/2 A' D^-1/2 (h @ W) + b
Aggregation commutes with the dense transform, so every layer runs
"transform first": z = dinv * (h @ W) (node-wise scale), then an
unweighted gather/segment-sum over the fixed edge list, then a second
dinv scale, bias and (optionally) relu.

Device mapping (nodes sharded 12500/core, padded to 12544 = 128*98):
  * z stored as fp16 256B rows [N, 128] in DRAM; AllGather replicates.
  * Edges sharded by dst core, sorted by (tile-group, src-quartile,
    tile).  Per (group, quartile) one dma_gather on SWDGE queue q with
    EXACT per-core row counts: idx arrays are -1-padded at the tail and
    the runtime count register is reg_load'ed from a per-core counts
    tensor (trailing -1 idxs generate no descriptors - Pool desc-gen
    at ~2.3ns/row is the bottleneck resource, so no padded rows).
  * Segments are unaligned inside a call: chunk boundaries sit on a
    fixed 128-grid and a chunk spanning two dst tiles is matmul'ed into
    both tiles' PSUM with complementary one-hot selectors.
  * Selectors are generated ON-CHIP on the (otherwise idle) DVE:
    sel[p, ci*128+j] = (p_dst[p, ci] == iota[j]) via a broadcast
    is_equal; only the tiny p_dst table (pad=255) is resident.  This
    removes ~61MB/layer of selector DMA traffic.
  * Self loops never enter the edge list: evac computes
    h = dinv*(agg + z_local) + b via two fused scalar_tensor_tensor
    DVE ops reading the PSUM directly.
  * z for the next layer is computed per-tile right after evac
    (transpose + fp16 matmul), so the AllGather fires immediately
    after the last tile's aggregation.
"""

import math
from collections import deque

import numpy as np

import concourse.bacc as bacc
import concourse.bass as bass
import concourse.mybir as mybir
import concourse.tile as tile
from concourse.bass_utils import run_bass_kernel_spmd
from concourse.masks import make_identity

P = 128
NC = 8
NQ = 4          # SWDGE queues == src-gid quartiles (core pairs)
G = 2           # dst tiles per gather call
GT_BUFS = 4
SEL_BUFS = 4

# layer schedule: (F_in, F_out, relu)
LAYERS = [(88, 65, True), (65, 50, False), (50, 65, True), (65, 88, False)]
FMAX = 128      # fp16 z-table row width (256B rows)

last_results = None


def _build_host_data(x, Ws, bs, edge_index, n_real, T):
    npc_real = n_real // NC
    npc = P * T
    quart = NC * npc // NQ
    assert quart - 1 <= 32767
    NG = (T + G - 1) // G
    assert NG * G == T
    NCALLS = NG * NQ

    src = edge_index[0].astype(np.int64)
    dst = edge_index[1].astype(np.int64)
    deg = np.bincount(dst, minlength=n_real).astype(np.float32) + 1.0  # + self loop

    c_dst = dst // npc_real
    r_dst = dst - c_dst * npc_real
    t_dst = r_dst % T
    p_dst = r_dst // T
    c_src = src // npc_real
    gid_src = c_src * npc + (src - c_src * npc_real)
    q_src = gid_src // quart
    loc_src = (gid_src - q_src * quart).astype(np.int64)
    g_dst = t_dst // G
    tig = t_dst - g_dst * G

    # counts [NC, NG, G, NQ]
    cnt = np.zeros(NC * NG * G * NQ, np.int64)
    ckey = ((c_dst * NG + g_dst) * G + tig) * NQ + q_src
    np.add.at(cnt, ckey, 1)
    cnt = cnt.reshape(NC, NG, G, NQ)

    cnt_call = cnt.sum(axis=2)                       # [NC, NG, NQ]
    NCH = -(-cnt_call.max(axis=0) // P)              # [NG, NQ] chunks per call
    nidx = NCH * P
    assert nidx.max() <= 16000

    # per-core segment boundaries inside a call
    cum = np.cumsum(cnt, axis=2)                     # [NC, NG, G, NQ]
    seg_start = np.zeros_like(cnt)
    seg_start[:, :, 1:, :] = cum[:, :, :-1, :]
    seg_end = cum
    ch_lo = (seg_start // P).min(axis=0)             # [NG, G, NQ]
    ch_hi = (-(-seg_end // P)).max(axis=0)           # [NG, G, NQ]
    nch_t = np.maximum(ch_hi - ch_lo, 0)             # chunks per (tile, q)
    ncht = nch_t.sum(axis=2).reshape(T)              # [T] selector chunks per tile
    pcol_base = np.zeros(T + 1, np.int64)
    np.cumsum(ncht, out=pcol_base[1:])
    PCOLS = int(pcol_base[-1])

    icol_off = np.zeros((NG, NQ), np.int64)          # idx col16 offset per call
    acc = 0
    for g in range(NG):
        for q in range(NQ):
            icol_off[g, q] = acc
            acc += int(nidx[g, q]) // 16
    ICOLS = int(acc)
    tot_idx = ICOLS * 16

    # edge order: (core, group, quartile, tile-in-group); stable
    skey = ((c_dst * NG + g_dst) * NQ + q_src) * G + tig
    order = np.argsort(skey, kind="stable")
    # rank within (c, g, q): position inside the call
    gkey = (c_dst * NG + g_dst) * NQ + q_src
    gkey_s = gkey[order]
    first = np.ones(len(gkey_s), bool)
    first[1:] = gkey_s[1:] != gkey_s[:-1]
    grp_first = np.where(first)[0]
    grp_id = np.cumsum(first) - 1
    pos = np.arange(len(gkey_s)) - grp_first[grp_id]

    t_s = t_dst[order]
    g_s = g_dst[order]
    tig_s = tig[order]
    q_s = q_src[order]
    c_s = c_dst[order]
    p_s = p_dst[order]
    loc_s = loc_src[order]

    ci = pos // P
    col = (
        pcol_base[t_s]
        + np.concatenate([np.zeros((NG, G, 1), np.int64), np.cumsum(nch_t, axis=2)], 2)[
            g_s, tig_s, q_s
        ]
        + (ci - ch_lo[g_s, tig_s, q_s])
    )
    ipos = icol_off[g_s, q_s] * 16 + pos

    per_core = []
    for c in range(NC):
        m = c_s == c
        vals = np.full(tot_idx, -1, np.int64)
        vals[ipos[m]] = loc_s[m]
        wrapped = vals.reshape(-1, 16).T.astype(np.int16)
        idx_all = np.tile(wrapped, (8, 1))           # [128, ICOLS]

        pdst = np.full((P, PCOLS), 255.0, np.float16)
        pdst[pos[m] % P, col[m]] = p_s[m].astype(np.float16)

        cnts = cnt_call[c].reshape(-1).astype(np.int32)[None, :]   # [1, NCALLS]

        xs = np.zeros((npc, x.shape[1]), np.float32)
        xs[:npc_real] = x[c * npc_real : (c + 1) * npc_real]
        degs = np.full(npc, 1.0e30, np.float32)
        degs[:npc_real] = deg[c * npc_real : (c + 1) * npc_real]
        x_pt = xs.reshape(P, T, x.shape[1]).reshape(P, T * x.shape[1])
        deg_pt = degs.reshape(P, T)

        inp = {"x": x_pt, "deg": deg_pt, "idx": idx_all, "pdst": pdst, "cnt": cnts}
        for li in range(len(LAYERS)):
            inp[f"W{li + 1}"] = Ws[li].astype(np.float16)
            inp[f"b{li + 1}"] = np.tile(bs[li].reshape(1, -1), (P, 1))
        per_core.append(inp)

    sched = {
        "T": T,
        "NG": NG,
        "NCALLS": NCALLS,
        "npc": npc,
        "npc_real": npc_real,
        "quart": quart,
        "NCH": NCH,
        "nidx": nidx,
        "ch_lo": ch_lo,
        "nch_t": nch_t,
        "ncht": ncht,
        "pcol_base": pcol_base,
        "icol_off": icol_off,
        "ICOLS": ICOLS,
        "PCOLS": PCOLS,
        "fin0": x.shape[1],
    }
    return per_core, sched


def _build_program(sched):
    T = sched["T"]
    NG = sched["NG"]
    NCALLS = sched["NCALLS"]
    npc = sched["npc"]
    quart = sched["quart"]
    NCH = sched["NCH"]
    nidx = sched["nidx"]
    ch_lo = sched["ch_lo"]
    nch_t = sched["nch_t"]
    ncht = sched["ncht"]
    pcol_base = sched["pcol_base"]
    icol_off = sched["icol_off"]
    ICOLS = sched["ICOLS"]
    PCOLS = sched["PCOLS"]
    fin0 = sched["fin0"]
    fout_last = LAYERS[-1][1]
    n_pad_total = NC * npc

    f32, f16 = mybir.dt.float32, mybir.dt.float16
    i16, i32 = mybir.dt.int16, mybir.dt.int32
    AF = mybir.ActivationFunctionType
    ALU = mybir.AluOpType

    nc = bacc.Bacc(
        "TRN2",
        target_bir_lowering=False,
        debug=False,
        num_devices=NC,
        num_swdge_queues=NQ,
    )

    x_t = nc.dram_tensor("x", [P, T * fin0], f32, kind="ExternalInput")
    deg_t = nc.dram_tensor("deg", [P, T], f32, kind="ExternalInput")
    idx_t = nc.dram_tensor("idx", [P, ICOLS], i16, kind="ExternalInput")
    pdst_t = nc.dram_tensor("pdst", [P, PCOLS], f16, kind="ExternalInput")
    cnt_t = nc.dram_tensor("cnt", [1, NCALLS], i32, kind="ExternalInput")
    W_ts, b_ts = [], []
    for li, (fi, fo, _) in enumerate(LAYERS):
        W_ts.append(nc.dram_tensor(f"W{li + 1}", [fi, fo], f16, kind="ExternalInput"))
        b_ts.append(nc.dram_tensor(f"b{li + 1}", [P, fo], f32, kind="ExternalInput"))
    out_t = nc.dram_tensor("out", [P, T * fout_last], f32, kind="ExternalOutput")

    zloc = nc.dram_tensor("zloc", [P, T * FMAX], f16)
    zfull = nc.dram_tensor("zfull", [n_pad_total, FMAX], f16, addr_space="Shared")

    with tile.TileContext(nc) as tc:
        with (
            tc.tile_pool(name="const", bufs=1) as cpool,
            tc.tile_pool(name="hbuf", bufs=1) as hpool,
            tc.tile_pool(name="zbuf", bufs=1) as zpool,
            tc.tile_pool(name="work", bufs=3) as wpool,
            tc.tile_pool(name="selp", bufs=SEL_BUFS) as selpool,
            tc.tile_pool(name="gt", bufs=GT_BUFS) as gtpool,
            tc.tile_pool(name="ps", bufs=2, space="PSUM") as pspool,
            tc.tile_pool(name="agg", bufs=4, space="PSUM") as aggpool,
        ):
            ident32 = cpool.tile([P, P], f32)
            make_identity(nc, ident32[:])
            ident16 = cpool.tile([P, P], f16)
            make_identity(nc, ident16[:])

            iota_i = cpool.tile([P, P], i32)
            nc.gpsimd.iota(iota_i[:], pattern=[[1, P]], base=0, channel_multiplier=0)
            iota = cpool.tile([P, P], f16)
            nc.scalar.activation(iota[:], iota_i[:], AF.Copy)

            idx_sb = cpool.tile([P, ICOLS], i16)
            nc.sync.dma_start(out=idx_sb[:], in_=idx_t[:])
            pdst_sb = cpool.tile([P, PCOLS], f16)
            nc.sync.dma_start(out=pdst_sb[:], in_=pdst_t[:])
            cnt_sb = cpool.tile([1, NCALLS], i32)
            nc.sync.dma_start(out=cnt_sb[:], in_=cnt_t[:])

            deg_sb = cpool.tile([P, T], f32)
            nc.sync.dma_start(out=deg_sb[:], in_=deg_t[:])
            dinv = cpool.tile([P, T], f32)
            nc.scalar.activation(dinv[:], deg_sb[:], AF.Sqrt)
            nc.vector.reciprocal(dinv[:], dinv[:])

            Wsb, bsb = [], []
            for li, (fi, fo, _) in enumerate(LAYERS):
                w = cpool.tile([fi, fo], f16, tag=f"w{li}")
                nc.sync.dma_start(out=w[:], in_=W_ts[li][:])
                b = cpool.tile([P, fo], f32, tag=f"bi{li}")
                nc.sync.dma_start(out=b[:], in_=b_ts[li][:])
                Wsb.append(w)
                bsb.append(b)

            hbuf = hpool.tile([P, T, 88], f16)
            zbuf = zpool.tile([P, T, FMAX], f16)
            nc.vector.memset(zbuf[:], 0.0)

            cregs = [nc.gpsimd.alloc_register(f"cntq{q}") for q in range(NQ)]

            def emit_z(t, li):
                """z_{li} tile t from hbuf (li>=1) or x (li==0), into zbuf."""
                fi, fo, _ = LAYERS[li]
                if li == 0:
                    xt = wpool.tile([P, fin0], f32, tag="xt")
                    nc.sync.dma_start(out=xt[:], in_=x_t[:, t * fin0 : (t + 1) * fin0])
                    h_tile = xt[:, :fi]
                    ident = ident32
                    tdt = f32
                else:
                    h_tile = hbuf[:, t, :fi]
                    ident = ident16
                    tdt = f16
                tp = pspool.tile([fi, P], tdt, tag="tps")
                nc.tensor.transpose(out=tp[:], in_=h_tile, identity=ident[:])
                ht = wpool.tile([fi, P], f16, tag="ht")
                nc.scalar.activation(ht[:], tp[:], AF.Copy)
                zp = pspool.tile([P, fo], f32, tag="zps")
                nc.tensor.matmul(
                    out=zp[:], lhsT=ht[:], rhs=Wsb[li][:], start=True, stop=True
                )
                nc.scalar.activation(
                    zbuf[:, t, :fo], zp[:], AF.Copy, scale=dinv[:, t : t + 1]
                )

            # ---- layer 0 messages from x ----
            for t in range(T):
                emit_z(t, 0)

            for li, (fi, fo, do_relu) in enumerate(LAYERS):
                nc.sync.dma_start(
                    out=zloc[:], in_=zbuf[:].rearrange("p t f -> p (t f)")
                )
                nc.gpsimd.collective_compute(
                    "AllGather",
                    mybir.AluOpType.bypass,
                    ins=[zloc[:]],
                    outs=[zfull[:]],
                    replica_groups=[list(range(NC))],
                )

                pending_z = deque()
                gts = {}
                for g in range(NG):
                    tiles = list(range(g * G, (g + 1) * G))
                    # selector generation (DVE) for this group's tiles
                    sels = {}
                    for t in tiles:
                        k = int(ncht[t])
                        if k == 0:
                            continue
                        sel = selpool.tile([P, k * P], f16, tag="sel")
                        pb = int(pcol_base[t])
                        nc.vector.tensor_tensor(
                            out=sel[:].rearrange("p (c j) -> p c j", c=k),
                            in0=pdst_sb[:, pb : pb + k]
                            .unsqueeze(2)
                            .broadcast_to([P, k, P]),
                            in1=iota[:].unsqueeze(1).broadcast_to([P, k, P]),
                            op=ALU.is_equal,
                        )
                        sels[t] = sel
                    # gathers (Pool): exact per-core counts
                    for q in range(NQ):
                        nch = int(NCH[g, q])
                        if nch == 0:
                            gts[(g, q)] = None
                            continue
                        gt = gtpool.tile([P, nch, FMAX], f16, tag=f"gt{q}")
                        if li * NG + g < GT_BUFS:
                            nc.vector.memset(gt[:], 0.0)
                        call = g * NQ + q
                        nc.gpsimd.reg_load(cregs[q], cnt_sb[0:1, call : call + 1])
                        ic = int(icol_off[g, q])
                        nc.gpsimd.dma_gather(
                            out_ap=gt[:],
                            in_ap=zfull[q * quart : (q + 1) * quart, :],
                            idxs_ap=idx_sb[:, ic : ic + int(nidx[g, q]) // 16],
                            num_idxs=int(nidx[g, q]),
                            num_idxs_reg=cregs[q],
                            elem_size=FMAX,
                            single_packet=False,
                            queue_num=q,
                        )
                        gts[(g, q)] = gt
                    # aggregation + evac for this group's tiles
                    for t in tiles:
                        tot = int(ncht[t])
                        if tot == 0:
                            continue
                        tg = t - g * G
                        agg = aggpool.tile([P, fo], f32, tag="agg")
                        done = 0
                        scol = 0
                        sel = sels[t]
                        for q in range(NQ):
                            k = int(nch_t[g, tg, q])
                            lo = int(ch_lo[g, tg, q])
                            gt = gts[(g, q)]
                            for j in range(k):
                                nc.tensor.matmul(
                                    out=agg[:],
                                    lhsT=sel[:, scol * P : (scol + 1) * P],
                                    rhs=gt[:, lo + j, :fo],
                                    start=(done == 0),
                                    stop=(done == tot - 1),
                                )
                                scol += 1
                                done += 1
                        # evac: h = dinv*(agg + z_self) + b  [+ relu]
                        # t1 = (z_self * dinv) + b ; h = (agg * dinv) + t1
                        t1 = wpool.tile([P, fo], f32, tag="ev1")
                        nc.vector.scalar_tensor_tensor(
                            out=t1[:],
                            in0=zbuf[:, t, :fo],
                            scalar=dinv[:, t : t + 1],
                            in1=bsb[li][:],
                            op0=ALU.mult,
                            op1=ALU.add,
                        )
                        if li < len(LAYERS) - 1:
                            if do_relu:
                                t2 = wpool.tile([P, fo], f32, tag="ev2")
                                nc.vector.scalar_tensor_tensor(
                                    out=t2[:],
                                    in0=agg[:],
                                    scalar=dinv[:, t : t + 1],
                                    in1=t1[:],
                                    op0=ALU.mult,
                                    op1=ALU.add,
                                )
                                nc.scalar.activation(hbuf[:, t, :fo], t2[:], AF.Relu)
                            else:
                                nc.vector.scalar_tensor_tensor(
                                    out=hbuf[:, t, :fo],
                                    in0=agg[:],
                                    scalar=dinv[:, t : t + 1],
                                    in1=t1[:],
                                    op0=ALU.mult,
                                    op1=ALU.add,
                                )
                            pending_z.append(t)
                        else:
                            ost = wpool.tile([P, fo], f32, tag="ost")
                            nc.vector.scalar_tensor_tensor(
                                out=ost[:],
                                in0=agg[:],
                                scalar=dinv[:, t : t + 1],
                                in1=t1[:],
                                op0=ALU.mult,
                                op1=ALU.add,
                            )
                            nc.sync.dma_start(
                                out=out_t[:, t * fo : (t + 1) * fo], in_=ost[:]
                            )
                    # fused z for the previous group's tiles (lag keeps PE fed)
                    if li < len(LAYERS) - 1 and g >= 1:
                        while len(pending_z) > G:
                            emit_z(pending_z.popleft(), li + 1)
                if li < len(LAYERS) - 1:
                    while pending_z:
                        emit_z(pending_z.popleft(), li + 1)

    nc.compile()
    return nc


def kernel(x, W1, b1, W2, b2, W3, b3, W4, b4, edge_index):
    global last_results
    x = np.asarray(x, np.float32)
    edge_index = np.asarray(edge_index)
    n_real = x.shape[0]
    T = math.ceil(n_real / NC / P)  # 98 for 100000
    Ws = [np.asarray(w, np.float32) for w in (W1, W2, W3, W4)]
    bs = [np.asarray(b, np.float32) for b in (b1, b2, b3, b4)]

    per_core, sched = _build_host_data(x, Ws, bs, edge_index, n_real, T)
    nc = _build_program(sched)

    res = run_bass_kernel_spmd(nc, per_core, list(range(NC)))
    last_results = res

    npc_real = sched["npc_real"]
    fo = LAYERS[-1][1]
    out = np.empty((n_real, fo), np.float32)
    for c in range(NC):
        o = res.results[c]["out"].reshape(P, T, fo)
        out[c * npc_real : (c + 1) * npc_real] = o.reshape(P * T, fo)[:npc_real]
    return out
